# revision 45
# baseline (speedup 1.0000x reference)
"""Trainium2 Bass kernel for nn_ActionSmoothingLoss.

Math (per row y of previous_actions, x = segmented log_softmax(current_action)):
    e = exp(y)                       (no max-subtraction: |y| <= ~5.5, safe in f32)
    Z_j = sum_{i in seg j} e_i
    S_j = sum_{i in seg j} e_i * (y_i - x_i)
    loss = (1/W) * sum_rows sum_j inv_n_j * [ S_j / Z_j - log Z_j ]

Sharding: data-parallel over W across 8 cores; x replicated; partial sums
(per-partition accumulators) gathered and combined on host in float64.

Device pipeline per tile [128 partitions, rt rows x 68] with the tile
schedule rt = [64, 64, 128, 128, 128] (small first for a short DMA ramp;
large after — per-tile fixed costs are ~3.5us of instr overhead + sems):
    ScalarE: e = exp(y) -> bf16; grouped per-inv_n Ln(Z) accums into accB.
    DVE:     d = y - x          (x via 0-stride broadcast AP, bf16 out)
             cums = MUL_CUMSUM(e, d)   <- custom DVE op: prefix-sum of the
                    product e*d in one 1x pass (fuses multiply + S-reduce)
             smp  = cums sampled at the 6 segment-end columns of each row
                    (3 strided copies: end cols {2,5},{9,34,59},{67})
             S    = adjacent-difference of smp  (exact per-(row,seg) sums:
                    the cumsum is continuous across rows, so diffs of
                    consecutive segment-end samples telescope correctly)
             Z    = per-segment tensor_reduces over e (adjacent equal-n
                    segments share one reduce via 4D APs: 4 instrs)
             rzi  = reciprocal_approx_fast(Z) * inv_n_broadcast
             stt  = S * rzi with accum into accA (1 instr)
Measured: 147.3us HW (baseline was 214us harness / 180.9us traced); DVE is
~90% busy at its floor of 3 full-tile 1x passes (sub, cumsum, Z-reduce).
Measured dead ends: GpSimd offload (shares DVE's SBUF port: 223us),
ScalarE sampling (cross-engine SBUF contention: 178us), PE/PSUM subtract
(per-matmul LDWEIGHTS + PSUM ping-pong lockstep: 152us), bf16 reduces
(tensor_reduce has only a 1x uop).
"""

import sys

sys.path.insert(0, "/opt/trn_rl_repo")

import numpy as np

NVEC = (3, 3, 4, 25, 25, 8)
OFFS = (0, 3, 6, 10, 35, 60)
ENDS = (2, 5, 9, 34, 59, 67)  # inclusive end column of each segment
A = 68
P = 128
N_CORES = 8
W_FULL = 524288
W_CORE = W_FULL // N_CORES  # 65536
R = 64                      # rows per partition per tile
F = R * A                   # 4352 free elems per tile
T = W_CORE // (P * R)       # 8 tiles per core

_PROGRAM_CACHE = {}
_MUL_CUMSUM = None


def _register_mul_cumsum():
    """Register the MUL_CUMSUM_ANT custom DVE op (out = cumsum(in0*in1) along
    the free dim, fp32 state). Uses the documented extension point
    (dve_ops.OPS); the uop table ships inside the NEFF so no firmware change
    is involved. Idempotent."""
    global _MUL_CUMSUM
    if _MUL_CUMSUM is not None:
        return _MUL_CUMSUM
    import concourse.dve_ops as dve_ops_mod
    from concourse.dve_spec import Spec, Src0, Src1, AluOp, scan, lower
    from concourse.dve_uop import DveOpSpec

    NAME = "MUL_CUMSUM_ANT"
    for op in dve_ops_mod.OPS:
        if op.name == NAME:
            _MUL_CUMSUM = op
            return op

    def _ref(in0, in1, s0, s1, imm2):
        p = in0.shape[0]
        prod = (np.asarray(in0, np.float32).reshape(p, -1)
                * np.asarray(in1, np.float32).reshape(p, -1)).astype(np.float32)
        return np.cumsum(prod, axis=-1, dtype=np.float32)

    spec = Spec(body=scan(AluOp.ADD, Src0 * Src1), reference=_ref)
    row = dve_ops_mod._CUSTOM_DVE_ROW_BASE + len(dve_ops_mod.OPS)
    assert row < 0x20
    shas = {}
    for ver in ("v3",):
        s = DveOpSpec(name=NAME, opcode=row, uops=lower(spec, ver=ver), rd1_en=True)
        shas[ver] = s.sha(ver)
    op = dve_ops_mod.DveOp(NAME, spec, subdim=False, uops_sha=shas)
    dve_ops_mod.OPS.append(op)
    dve_ops_mod._SUB_OPCODE_FOR_NAME[NAME] = row
    dve_ops_mod.CUSTOM_DVE_SPECS[NAME] = spec
    _MUL_CUMSUM = op
    return op


def build_program(w_core=W_CORE, r=R):
    import concourse.bass as bass
    import concourse.bacc as bacc
    import concourse.mybir as mybir
    from concourse import tile

    mul_cumsum = _register_mul_cumsum()

    f32 = mybir.dt.float32
    bf16 = mybir.dt.bfloat16
    # Tile schedule: small tiles first (short DMA ramp), large tiles after
    # (per-tile fixed costs — instruction overheads + semaphores — are ~3.8us,
    # so fewer/larger tiles win once the pipeline is primed).
    rows_pp = w_core // P  # rows per partition
    if rows_pp >= 384 and (rows_pp - 128) % 128 == 0:
        RS = [64, 64] + [128] * ((rows_pp - 128) // 128)
    else:
        RS = [64] * (rows_pp // 64)
    assert sum(RS) == rows_pp
    r_max = max(RS)
    Fmax = r_max * A
    Tt = len(RS)

    Exp = mybir.ActivationFunctionType.Exp
    Ln = mybir.ActivationFunctionType.Ln
    sub_op = mybir.AluOpType.subtract
    mult_op = mybir.AluOpType.mult
    add_op = mybir.AluOpType.add
    AX = mybir.AxisListType.X

    nc = bacc.Bacc(None, target_bir_lowering=False)
    pa = nc.dram_tensor("pa", [w_core, A], f32, kind="ExternalInput")
    # xb carries x broadcast (cols 0..67) plus the 6 inv_n values (68..73).
    xb = nc.dram_tensor("xb", [P, A + 6], f32, kind="ExternalInput")
    acc_a = nc.dram_tensor("acc_a", [P, Tt], f32, kind="ExternalOutput")
    acc_b = nc.dram_tensor("acc_b", [P, Tt * 4], f32, kind="ExternalOutput")

    # [P, rows_pp*A] view: partition p holds its rows contiguously; tile t
    # takes the next rt rows of every partition.
    pav = pa.rearrange("(p q) a -> p (q a)", p=P)

    with tile.TileContext(nc) as tc:
        with tc.tile_pool(name="io", bufs=2) as io, \
             tc.tile_pool(name="wk", bufs=2) as wk, \
             tc.tile_pool(name="cm", bufs=1) as cm, \
             tc.tile_pool(name="sm", bufs=1) as sm, \
             tc.tile_pool(name="zp", bufs=2) as zp, \
             tc.tile_pool(name="ps", bufs=1) as ps:
            xbt = ps.tile([P, A + 6], f32)
            nc.sync.dma_start(xbt[:], xb[:], single_packet=True)
            accA = ps.tile([P, Tt], f32)
            accB = ps.tile([P, Tt * 4], f32)
            row0 = 0
            for t, rt in enumerate(RS):
                Ft = rt * A
                S6 = 6 * rt
                H = Ft // 2
                src = pav[:, row0 * A:(row0 + rt) * A]
                row0 += rt
                y = io.tile([P, Fmax], f32, tag="y")
                # e/d in bf16: halves SBUF streaming (cross-engine bank
                # contention measurably inflates DVE op durations). x stays
                # fp32 — its quantization error is common-mode across rows and
                # would bias the loss (~1.4e-3); e/d rounding averages out.
                e = wk.tile([P, Fmax], bf16, tag="e")
                d = wk.tile([P, Fmax], bf16, tag="d")
                cums = cm.tile([P, Fmax], f32, tag="cums")
                # x read via 0-stride broadcast AP (no materialized xbb tile).
                # Tile 0 is sliced in quarters (shorter DMA ramp before the
                # first DVE work); later tiles use halves for exp and a single
                # sub on the big tiles.
                if t == 0:
                    Q = Ft // 4
                    ex_sl = sub_sl = tuple(
                        (q * Q, (q + 1) * Q) for q in range(4))
                    nc.sync.dma_start(y[:, :Q], src[:, :Q])
                    nc.sync.dma_start(y[:, Q:H], src[:, Q:H])
                    nc.sync.dma_start(y[:, H:H + Q], src[:, H:H + Q])
                    nc.sync.dma_start(y[:, H + Q:Ft], src[:, H + Q:])
                else:
                    ex_sl = ((0, H), (H, Ft))
                    sub_sl = ex_sl if rt < r_max else ((0, Ft),)
                    nc.sync.dma_start(y[:, :H], src[:, :H])
                    nc.sync.dma_start(y[:, H:Ft], src[:, H:])
                for h0, h1 in ex_sl:
                    nc.scalar.activation(e[:, h0:h1], y[:, h0:h1], Exp)
                e3 = e[:, :Ft].rearrange("p (r a) -> p r a", r=rt)
                d3 = d[:, :Ft].rearrange("p (r a) -> p r a", r=rt)
                # Z double-buffered: ScalarE's Ln reads Z(t), so a single
                # buffer would make tile t+1's Z-reduces wait on ScalarE
                # (cross-engine WAR sems on the DVE queue every tile).
                Z = zp.tile([P, 6 * r_max], f32, tag="Z")
                Z3 = Z[:, :S6].rearrange("p (r s) -> p r s", s=6)

                def emit_subs():
                    for h0, h1 in sub_sl:
                        nc.vector.tensor_tensor(
                            d[:, h0:h1].rearrange("p (r a) -> p r a", a=A),
                            y[:, h0:h1].rearrange("p (r a) -> p r a", a=A),
                            xbt[:, :A].unsqueeze(1).broadcast_to(
                                (P, (h1 - h0) // A, A)),
                            op=sub_op)

                def emit_zreds(r0, r1):
                    # Adjacent equal-size segments ({0,1}: n=3, {3,4}: n=25)
                    # share one reduce via a 4D [P, r, 2, n] AP.
                    for j0, k, o, n in ((0, 2, 0, 3), (2, 1, 6, 4),
                                        (3, 2, 10, 25), (5, 1, 60, 8)):
                        nc.vector.tensor_reduce(
                            Z3[:, r0:r1, j0:j0 + k],
                            e3[:, r0:r1, o:o + k * n].rearrange(
                                "p r (k n) -> p r k n", k=k),
                            axis=AX, op=add_op)

                if t == 0:
                    # Tile 0: the subs wait on the slow small-line xbt DMA
                    # (~10us); the Z-reduces need only e, so run them first
                    # (per row-half, each needing only half of e) to cover
                    # that wait.
                    emit_zreds(0, rt // 2)
                    emit_zreds(rt // 2, rt)
                    emit_subs()
                else:
                    emit_subs()
                    emit_zreds(0, rt)
                # cums = running sum of e*d over the flat [r*A] stream.
                nc.vector._custom_dve(
                    mul_cumsum, out=cums[:, :Ft], in0=e3, in1=d3)
                cums3 = cums[:, :Ft].rearrange("p (r a) -> p r a", r=rt)
                # Sample the cumsum at each segment-end column; j-innermost
                # layout so one adjacent-difference yields every segment sum.
                # End cols {2,5}, {9,34,59}, {67} have affine strides, so three
                # strided copies cover all six.
                smp = sm.tile([P, 6 * r_max], f32, tag="smp")
                smp3 = smp[:, :S6].rearrange("p (r s) -> p r s", s=6)
                nc.vector.tensor_copy(smp3[:, :, 0:2], cums3[:, :, 2:6:3])
                nc.vector.tensor_copy(smp3[:, :, 2:5], cums3[:, :, 9:60:25])
                nc.vector.tensor_copy(smp3[:, :, 5:6], cums3[:, :, 67:68])
                Sg = sm.tile([P, 6 * r_max], f32, tag="Sg")
                nc.vector.tensor_copy(Sg[:, 0:1], smp[:, 0:1])
                nc.vector.tensor_tensor(
                    Sg[:, 1:S6], smp[:, 1:S6], smp[:, :S6 - 1], op=sub_op)
                rz = sm.tile([P, 6 * r_max], f32, tag="rz")
                nc.vector.reciprocal_approx_fast(rz[:, :S6], Z[:, :S6])
                # Fold inv_n into the reciprocal so one stt covers all 6 segs.
                # inv_n is read via a 0-stride broadcast AP (a materialized
                # copy would be hoisted by the scheduler ahead of tile 0 and
                # stall the DVE queue on the slow small-line xbt DMA).
                rzi = sm.tile([P, 6 * r_max], f32, tag="rzi")
                nc.vector.tensor_tensor(
                    rzi[:, :S6].rearrange("p (r s) -> p r s", s=6),
                    rz[:, :S6].rearrange("p (r s) -> p r s", s=6),
                    xbt[:, A:A + 6].unsqueeze(1).broadcast_to((P, rt, 6)),
                    op=mult_op)
                # Ln accumulation grouped by equal inv_n (segments {0,1}, {2},
                # {3,4}, {5}) — 4 ScalarE instrs instead of 6.
                L = sm.tile([P, 6 * r_max], f32, tag="L")
                for g, (j0, k) in enumerate(((0, 2), (2, 1), (3, 2), (5, 1))):
                    nc.scalar.activation(
                        L[:, j0 * rt:(j0 + k) * rt].rearrange(
                            "p (r s) -> p r s", s=k),
                        Z3[:, :, j0:j0 + k], Ln,
                        accum_out=accB[:, t * 4 + g: t * 4 + g + 1])
                to = sm.tile([P, 6 * r_max], f32, tag="to")
                nc.vector.scalar_tensor_tensor(
                    out=to[:, :S6],
                    in0=Sg[:, :S6],
                    scalar=1.0,
                    in1=rzi[:, :S6],
                    op0=mult_op,
                    op1=mult_op,
                    accum_out=accA[:, t: t + 1])
            nc.sync.dma_start(acc_a[:], accA[:])
            nc.sync.dma_start(acc_b[:], accB[:])
    with _force_exp_ln_one_table_set():
        nc.compile()
    return nc, Tt


def _force_exp_ln_one_table_set():
    """Make the act-table pass map both Exp and Ln to
    natural_log_exp_and_others (otherwise it alternates exp_and_others /
    natural_log per tile: 14 ACT_TABLE_LOADs ~= 18us of ScalarE time)."""
    import contextlib
    import concourse.bacc as bacc_mod
    import concourse.mybir as mybir

    @contextlib.contextmanager
    def ctx():
        orig = bacc_mod.get_activation_tables

        def patched(arch):
            tables = {k: set(v) for k, v in orig(arch).items()}
            for name, funcs in tables.items():
                if name != "natural_log_exp_and_others":
                    funcs.discard(mybir.ActivationFunctionType.Exp)
                    funcs.discard(mybir.ActivationFunctionType.Ln)
            return tables

        bacc_mod.get_activation_tables = patched
        try:
            yield
        finally:
            bacc_mod.get_activation_tables = orig

    return ctx()


def _get_program():
    key = (W_CORE, R)
    if key not in _PROGRAM_CACHE:
        _PROGRAM_CACHE[key] = build_program(W_CORE, R)
    return _PROGRAM_CACHE[key]


def _host_x(current_action):
    """Segmented log_softmax of current_action in float64 on host."""
    ca = np.asarray(current_action, np.float64)
    x = np.empty(A, np.float64)
    for o, n in zip(OFFS, NVEC):
        seg = ca[o:o + n]
        m = seg.max()
        x[o:o + n] = seg - (m + np.log(np.exp(seg - m).sum()))
    return x


def combine_partials(results, w_full=W_FULL):
    """Combine per-core acc_a [P,T] (inv_n-weighted S/Z partials) and
    acc_b [P,T*4] (per-inv_n-group log-sums) into the scalar loss."""
    inv_g = np.asarray([1.0 / 3, 1.0 / 4, 1.0 / 25, 1.0 / 8], np.float64)
    total = 0.0
    for res in results:
        a = np.asarray(res["acc_a"], np.float64)
        b = np.asarray(res["acc_b"], np.float64)
        total += a.sum()  # inv_n already folded in on-device
        bt = b.reshape(P, -1, 4).sum(axis=(0, 1))  # [4] group log-sums
        total -= (bt * inv_g).sum()
    return np.float32(total / w_full)


def _make_xbt(current_action):
    """Host-side xb payload: x broadcast [P, 68] ++ inv_n [P, 6]."""
    x = _host_x(current_action).astype(np.float32)
    row = np.concatenate([x, (1.0 / np.asarray(NVEC, np.float32))])
    return np.broadcast_to(row, (P, A + 6)).copy()


def kernel(current_action, previous_actions):
    from concourse import bass_utils

    nc, _ = _get_program()
    xbt = _make_xbt(current_action)
    pa = np.ascontiguousarray(np.asarray(previous_actions, np.float32))
    assert pa.shape == (W_FULL, A)
    in_maps = [
        {"pa": pa[c * W_CORE:(c + 1) * W_CORE], "xb": xbt}
        for c in range(N_CORES)
    ]
    res = bass_utils.run_bass_kernel_spmd(
        nc, in_maps, core_ids=list(range(N_CORES)))
    return combine_partials(res.results)


if __name__ == "__main__":
    np.random.seed(0)
    ca = np.random.randn(A).astype(np.float32)
    pa = np.random.randn(W_FULL, A).astype(np.float32)
    print(kernel(ca, pa))
